# revision 9
# baseline (speedup 1.0000x reference)
"""AutoCorrelation kernel for Trainium2 (Bass/Tile), 8-core data parallel.

Math: the reference computes rfft over the zero-padded head dim (D=64 -> L=512),
multiplies conj(Q)*K, irffts, then MEANS over heads AND the whole lag axis.
Summing a circular correlation over all lags factorizes:
    sum_t corr[t] = (sum_d q[d]) * (sum_d k[d])
so  x_corr_mean[b,l] = 1/(H*L) * sum_h (sum_d q[b,l,h,:]) * (sum_d k[b,l,h,:]).
Then top-6 over l per batch, softmax, weighted sum of values rows -> [B,H,D].

Sharding: batch 16 -> 2 per core across 8 cores, no cross-core communication.

Schedule (one core): inputs stream via SWDGE DMAs that cast f32->bf16 in
flight (same HBM bytes, half the DVE reduce work at 2x 16-bit throughput;
verified: bf16 corr keeps the same top-6 sets with 9e-4 margins).  Batch 0
streams first and its whole pipeline hides under batch 1's streaming; batch
1's k arrives in single chunks with the last chunk split in half so the
final arrival-to-corr step is short.  DVE emission order is hand-interleaved
so batch-0's tail ops sit in the DMA-arrival gaps.  corr chunks are fused
multiply+reduce (tensor_tensor_reduce) then PE [128,1]->[1,128] column
transposes straight into a [1,512] PSUM row read by MAX8/FIND_INDEX8 (no
SBUF rake).  FIND writes indices directly into the uint32 32x32 staging
tile; one stream transpose puts the 6 gather row-ids on partitions; batch
1's +512 row offset rides the indirect DMA's element_offset.  The gather
casts f32->bf16 in flight and the exp-weight column is cast to bf16, so the
weighted sum is one single-pass bf16 matmul with UNNORMALIZED weights (ACT
exp emits its own sum via accum_out; 1/sum is folded into the PSUM->SBUF
output copy's scale).  A warm exp at entry (written to a live-but-unused
staging row so DCE keeps it) forces the ACT exp-table load before HBM
saturates.
"""

import numpy as np

import concourse.bass as bass
import concourse.mybir as mybir
import concourse.tile as tile
from concourse.masks import make_identity
from concourse.bass_utils import run_bass_kernel_spmd

B, L, H, D = 16, 512, 8, 64
HD = H * D                  # 512
NCORES = 8
BPC = B // NCORES           # 2 batches per core
ROWS = BPC * L              # 1024 rows of [HD] per core
P = 128
TPB = L // P                # 4 chunks per batch
KTOP = 6                    # k = int(log(512)) = 6
SCALE = 1.0 / (H * L)

_CACHE = {}

F32 = mybir.dt.float32
U32 = mybir.dt.uint32
BF16 = mybir.dt.bfloat16

import os
BF16_STREAM = os.environ.get("KBF16_STREAM", "1") == "1"   # SWDGE cast input stream
BF16_GATHER = os.environ.get("KBF16_GATHER", "1") == "1"   # bf16 gather + MM
ELEM_OFF = os.environ.get("KELEM_OFF", "1") == "1"         # element_offset on gather1
USE_TTR = os.environ.get("KTTR", "1") == "1"               # fused tensor_tensor_reduce


def _emit(tc, q, k, v, out):
    # out: single [BPC, HD] DRAM AP.
    nc = tc.nc
    from contextlib import ExitStack

    with ExitStack() as ctx:
        main = ctx.enter_context(tc.tile_pool(name="main", bufs=1))
        small = ctx.enter_context(tc.tile_pool(name="small", bufs=1))
        psum = ctx.enter_context(tc.tile_pool(name="psum", bufs=1, space="PSUM"))

        # Staging tiles for the 32x32 transposes, zeroed early on DVE (idle
        # until the first reduce) so transposes never read uninitialized SBUF.
        stage_u, stage_e = [], []
        for b in range(BPC):
            su = small.tile([32, 32], U32, name=f"su{b}", tag=f"su{b}")
            nc.vector.memset(su[:], 0)
            se = small.tile([32, 32], F32, name=f"se{b}", tag=f"se{b}")
            nc.vector.memset(se[:], 0.0)
            stage_u.append(su)
            stage_e.append(se)

        # Warm exp at entry: forces the ACT exp-table load before the input
        # stream saturates HBM.  Output goes to stage_e[0] row 0, cols 24:32
        # (live tile; lands in unused transpose rows, so DCE keeps it).
        nc.scalar.activation(
            out=stage_e[0][0:1, 24:32],
            in_=stage_e[0][0:1, 24:32],
            func=mybir.ActivationFunctionType.Exp,
            scale=0.0,
        )

        ident = small.tile([P, P], F32)
        make_identity(nc, ident[:])

        q3 = q.rearrange("(t p) m -> t p m", p=P)
        k3 = k.rearrange("(t p) m -> t p m", p=P)

        # ---- input DMAs: SWDGE (GpSimd) with f32->bf16 cast in flight.
        # Issue order = arrival order: batch 0 (coarse pieces) then batch 1
        # (fine pieces, last k chunk split in half). ----
        qt, kt = [], []
        piece_plans = [
            # (q pieces, k pieces, k half-split?)
            ([(0, 4)], [(0, 3), (3, 4)], False),
            ([(0, 2), (2, 4)], [(0, 2), (2, 3)], True),
        ]
        for b in range(BPC):
            t0 = b * TPB
            qp, kp, khalf = piece_plans[b]
            sdt = BF16 if BF16_STREAM else F32
            qt_b = main.tile([P, TPB, HD], sdt, tag=f"qt{b}")
            kt_b = main.tile([P, TPB, HD], sdt, tag=f"kt{b}")
            deng = nc.gpsimd if BF16_STREAM else nc.sync
            for lo, hi in qp:
                deng.dma_start(
                    out=qt_b[:, lo:hi, :],
                    in_=q3[t0 + lo : t0 + hi].rearrange("t p m -> p t m"),
                )
            for lo, hi in kp:
                deng.dma_start(
                    out=kt_b[:, lo:hi, :],
                    in_=k3[t0 + lo : t0 + hi].rearrange("t p m -> p t m"),
                )
            if khalf:
                for c0, c1 in [(0, HD // 2), (HD // 2, HD)]:
                    deng.dma_start(
                        out=kt_b[:, 3:4, c0:c1],
                        in_=k3[t0 + 3 : t0 + 4, :, c0:c1].rearrange(
                            "t p m -> p t m"
                        ),
                    )
            qt.append(qt_b)
            kt.append(kt_b)

        sq = [small.tile([P, TPB * H], F32, name=f"sq{b}", tag=f"sq{b}") for b in range(BPC)]
        sk = [small.tile([P, TPB * H], F32, name=f"sk{b}", tag=f"sk{b}") for b in range(BPC)]
        prod = [small.tile([P, TPB * H], F32, name=f"prod{b}", tag=f"prod{b}") for b in range(BPC)]
        corr_c = [small.tile([P, TPB], F32, name=f"corrc{b}", tag=f"corrc{b}") for b in range(BPC)]
        corr_ps = [psum.tile([1, L], F32, name=f"corrps{b}", tag=f"corrps{b}") for b in range(BPC)]
        maxv = [small.tile([1, 8], F32, name=f"maxv{b}", tag=f"maxv{b}") for b in range(BPC)]
        tu = [small.tile([32, 32], U32, name=f"tu{b}", tag=f"tu{b}") for b in range(BPC)]
        te = [small.tile([32, 32], F32, name=f"te{b}", tag=f"te{b}") for b in range(BPC)]
        teb = [small.tile([KTOP, 1], BF16 if BF16_GATHER else F32, name=f"teb{b}", tag=f"teb{b}") for b in range(BPC)]
        gath = [small.tile([KTOP, HD], BF16 if BF16_GATHER else F32, name=f"g{b}", tag=f"g{b}") for b in range(BPC)]
        s_t = [small.tile([1, 1], F32, name=f"s{b}", tag=f"s{b}") for b in range(BPC)]
        rs = [small.tile([1, 1], F32, name=f"rs{b}", tag=f"rs{b}") for b in range(BPC)]
        mm_ps = [psum.tile([1, HD], F32, name=f"mmps{b}", tag=f"mmps{b}") for b in range(BPC)]
        outt = [small.tile([1, HD], F32, name=f"outt{b}", tag=f"outt{b}") for b in range(BPC)]

        def rowsum(b, s_, t_, lo, hi):
            nc.vector.reduce_sum(
                out=s_[b][:, lo * H : hi * H],
                in_=t_[b][:, lo:hi, :].rearrange("p t (h d) -> p (t h) d", d=D),
                axis=mybir.AxisListType.X,
            )

        def rowsum_half(b, half):
            c0 = half * (HD // 2)
            nc.vector.reduce_sum(
                out=sk[b][:, 3 * H + 4 * half : 3 * H + 4 * (half + 1)],
                in_=kt[b][:, 3:4, c0 : c0 + HD // 2].rearrange(
                    "p t (h d) -> p (t h) d", d=D
                ),
                axis=mybir.AxisListType.X,
            )

        def corr_chunk(b, c):
            # fused: prod = sq*sk, corr_c[:, c] = sum(prod) in one DVE op,
            # then PE transposes the column into the [1, 512] PSUM corr row.
            if USE_TTR:
                nc.vector.tensor_tensor_reduce(
                    out=prod[b][:, c * H : (c + 1) * H],
                    in0=sq[b][:, c * H : (c + 1) * H],
                    in1=sk[b][:, c * H : (c + 1) * H],
                    scale=1.0,
                    scalar=0.0,
                    op0=mybir.AluOpType.mult,
                    op1=mybir.AluOpType.add,
                    accum_out=corr_c[b][:, c : c + 1],
                )
            else:
                nc.vector.tensor_mul(
                    prod[b][:, c * H : (c + 1) * H],
                    sq[b][:, c * H : (c + 1) * H],
                    sk[b][:, c * H : (c + 1) * H],
                )
                nc.vector.reduce_sum(
                    out=corr_c[b][:, c : c + 1],
                    in_=prod[b][:, c * H : (c + 1) * H],
                    axis=mybir.AxisListType.X,
                )
            nc.tensor.transpose(
                out=corr_ps[b][0:1, c * P : (c + 1) * P],
                in_=corr_c[b][:, c : c + 1],
                identity=ident[:],
            )

        def tail_head(b):
            nc.vector.max(out=maxv[b][:], in_=corr_ps[b][:])
            nc.vector.max_index(
                out=stage_u[b][0:1, 0:8],
                in_max=maxv[b][:],
                in_values=corr_ps[b][:],
            )
            if not ELEM_OFF and b:
                nc.vector.tensor_scalar_add(
                    stage_u[b][0:1, 0:8], stage_u[b][0:1, 0:8], b * L
                )
            nc.vector.transpose(out=tu[b][:], in_=stage_u[b][:])

        def tail_gather(b):
            nc.gpsimd.indirect_dma_start(
                out=gath[b][:],
                out_offset=None,
                in_=v,
                in_offset=bass.IndirectOffsetOnAxis(
                    ap=tu[b][0:KTOP, 0:1], axis=0
                ),
                element_offset=(b * L * HD) if ELEM_OFF else 0,
            )

        def tail_aux(b):
            # unnormalized softmax weights: e = exp(corr*SCALE); sum for free
            nc.scalar.activation(
                out=stage_e[b][0:1, 0:KTOP],
                in_=maxv[b][0:1, 0:KTOP],
                func=mybir.ActivationFunctionType.Exp,
                scale=SCALE,
                accum_out=s_t[b][:],
            )
            nc.vector.reciprocal(out=rs[b][:], in_=s_t[b][:])
            nc.vector.transpose(out=te[b][:], in_=stage_e[b][:])
            nc.vector.tensor_copy(teb[b][:], te[b][0:KTOP, 0:1])

        def tail_mm(b):
            nc.tensor.matmul(
                out=mm_ps[b][:],
                lhsT=teb[b][:],
                rhs=gath[b][:],
                start=True,
                stop=True,
            )
            nc.scalar.activation(
                out=outt[b][:],
                in_=mm_ps[b][:],
                func=mybir.ActivationFunctionType.Copy,
                scale=rs[b][0:1, 0:1],
            )
            nc.sync.dma_start(out=out[b : b + 1, :], in_=outt[b][:])

        # ---- hand-interleaved emission (per-engine program order) ----
        rowsum(0, sq, qt, 0, 4)          # DVE: batch-0 q (one piece)
        rowsum(0, sk, kt, 0, 3)          # DVE: batch-0 k piece a
        rowsum(0, sk, kt, 3, 4)          # DVE: batch-0 k piece b
        for c in range(TPB):
            corr_chunk(0, c)             # DVE ttr + PE transpose
        tail_head(0)                     # DVE max/find/transpose
        tail_gather(0)                   # GpSimd (queued after input issues)
        rowsum(1, sq, qt, 0, 2)          # DVE: batch-1 q pieces
        rowsum(1, sq, qt, 2, 4)
        tail_aux(0)                      # ACT exp + DVE recip/te/cast
        rowsum(1, sk, kt, 0, 2)          # DVE: batch-1 k as it lands
        rowsum(1, sk, kt, 2, 3)
        corr_chunk(1, 0)
        corr_chunk(1, 1)
        rowsum_half(1, 0)
        rowsum_half(1, 1)
        corr_chunk(1, 2)
        corr_chunk(1, 3)
        tail_head(1)
        tail_gather(1)
        tail_aux(1)
        tail_mm(0)                       # PE MMs last so they never block T1
        tail_mm(1)


def _build_bass():
    import concourse.bacc as bacc

    nc = bacc.Bacc(trn_type="TRN2", target_bir_lowering=False, debug=False)
    q = nc.dram_tensor("q", [ROWS, HD], mybir.dt.float32, kind="ExternalInput").ap()
    k = nc.dram_tensor("k", [ROWS, HD], mybir.dt.float32, kind="ExternalInput").ap()
    v = nc.dram_tensor("v", [ROWS, HD], mybir.dt.float32, kind="ExternalInput").ap()
    out = nc.dram_tensor(
        "out", [BPC, HD], mybir.dt.float32, kind="ExternalOutput"
    ).ap()
    with tile.TileContext(nc) as tc:
        _emit(tc, q, k, v, out)
    nc.compile()
    return nc


def _get_nc():
    if "nc" not in _CACHE:
        _CACHE["nc"] = _build_bass()
    return _CACHE["nc"]


def run_sharded(queries, keys, values, trace=False, **kw):
    """Shard over 8 cores, run, gather. Returns (out [16,8,64], BassKernelResults)."""
    nc = _get_nc()
    q = np.ascontiguousarray(np.asarray(queries, dtype=np.float32))
    k = np.ascontiguousarray(np.asarray(keys, dtype=np.float32))
    v = np.ascontiguousarray(np.asarray(values, dtype=np.float32))
    in_maps = []
    for c in range(NCORES):
        sl = slice(c * BPC, (c + 1) * BPC)
        in_maps.append(
            {
                "q": q[sl].reshape(ROWS, HD),
                "k": k[sl].reshape(ROWS, HD),
                "v": v[sl].reshape(ROWS, HD),
            }
        )
    res = run_bass_kernel_spmd(nc, in_maps, list(range(NCORES)), trace=trace, **kw)
    out = np.empty((B, H, D), dtype=np.float32)
    for c in range(NCORES):
        out[c * BPC : (c + 1) * BPC] = res.results[c]["out"].reshape(BPC, H, D)
    return out, res


def kernel(queries, keys, values, B=None, **_ignored):
    out, _ = run_sharded(queries, keys, values, trace=False)
    return out


# revision 10
# speedup vs baseline: 1.0932x; 1.0932x over previous
"""AutoCorrelation kernel for Trainium2 (Bass/Tile), 8-core data parallel.

Math: the reference computes rfft over the zero-padded head dim (D=64 -> L=512),
multiplies conj(Q)*K, irffts, then MEANS over heads AND the whole lag axis.
Summing a circular correlation over all lags factorizes:
    sum_t corr[t] = (sum_d q[d]) * (sum_d k[d])
so  x_corr_mean[b,l] = 1/(H*L) * sum_h (sum_d q[b,l,h,:]) * (sum_d k[b,l,h,:]).
Then top-6 over l per batch, softmax, weighted sum of values rows -> [B,H,D].

Sharding: batch 16 -> 2 per core across 8 cores, no cross-core communication.

Schedule (one core): inputs stream via SWDGE DMAs that cast f32->bf16 in
flight (same HBM bytes, half the DVE reduce work at 2x 16-bit throughput;
verified: bf16 corr keeps the same top-6 sets with 9e-4 margins).  Batch 0
streams first and its whole pipeline hides under batch 1's streaming; batch
1's k arrives in single chunks with the last chunk split in half so the
final arrival-to-corr step is short.  DVE emission order is hand-interleaved
so batch-0's tail ops sit in the DMA-arrival gaps.  corr chunks are fused
multiply+reduce (tensor_tensor_reduce) then PE [128,1]->[1,128] column
transposes straight into a [1,512] PSUM row read by MAX8/FIND_INDEX8 (no
SBUF rake).  FIND writes indices directly into the uint32 32x32 staging
tile; one stream transpose puts the 6 gather row-ids on partitions; batch
1's +512 row offset rides the indirect DMA's element_offset.  The gather
casts f32->bf16 in flight and the exp-weight column is cast to bf16, so the
weighted sum is one single-pass bf16 matmul with UNNORMALIZED weights (ACT
exp emits its own sum via accum_out; 1/sum is folded into the PSUM->SBUF
output copy's scale).  A warm exp at entry (written to a live-but-unused
staging row so DCE keeps it) forces the ACT exp-table load before HBM
saturates.
"""

import numpy as np

import concourse.bass as bass
import concourse.mybir as mybir
import concourse.tile as tile
from concourse.masks import make_identity
from concourse.bass_utils import run_bass_kernel_spmd

B, L, H, D = 16, 512, 8, 64
HD = H * D                  # 512
NCORES = 8
BPC = B // NCORES           # 2 batches per core
ROWS = BPC * L              # 1024 rows of [HD] per core
P = 128
TPB = L // P                # 4 chunks per batch
KTOP = 6                    # k = int(log(512)) = 6
SCALE = 1.0 / (H * L)

_CACHE = {}

F32 = mybir.dt.float32
U32 = mybir.dt.uint32
BF16 = mybir.dt.bfloat16

import os
BF16_STREAM = os.environ.get("KBF16_STREAM", "0") == "1"   # SWDGE cast input stream
BF16_GATHER = os.environ.get("KBF16_GATHER", "1") == "1"   # bf16 gather + MM
ELEM_OFF = os.environ.get("KELEM_OFF", "1") == "1"         # element_offset on gather1
USE_TTR = os.environ.get("KTTR", "0") == "1"  # WEDGES THE DEVICE - keep off               # fused tensor_tensor_reduce


def _emit(tc, q, k, v, out):
    # out: single [BPC, HD] DRAM AP.
    nc = tc.nc
    from contextlib import ExitStack

    with ExitStack() as ctx:
        main = ctx.enter_context(tc.tile_pool(name="main", bufs=1))
        small = ctx.enter_context(tc.tile_pool(name="small", bufs=1))
        psum = ctx.enter_context(tc.tile_pool(name="psum", bufs=1, space="PSUM"))

        # Staging tiles for the 32x32 transposes, zeroed early on DVE (idle
        # until the first reduce) so transposes never read uninitialized SBUF.
        stage_u, stage_e = [], []
        for b in range(BPC):
            su = small.tile([32, 32], U32, name=f"su{b}", tag=f"su{b}")
            nc.vector.memset(su[:], 0)
            se = small.tile([32, 32], F32, name=f"se{b}", tag=f"se{b}")
            nc.vector.memset(se[:], 0.0)
            stage_u.append(su)
            stage_e.append(se)

        # Warm exp at entry: forces the ACT exp-table load before the input
        # stream saturates HBM.  Output goes to stage_e[0] row 0, cols 24:32
        # (live tile; lands in unused transpose rows, so DCE keeps it).
        nc.scalar.activation(
            out=stage_e[0][0:1, 24:32],
            in_=stage_e[0][0:1, 24:32],
            func=mybir.ActivationFunctionType.Exp,
            scale=0.0,
        )

        ident = small.tile([P, P], F32)
        make_identity(nc, ident[:])

        q3 = q.rearrange("(t p) m -> t p m", p=P)
        k3 = k.rearrange("(t p) m -> t p m", p=P)

        # ---- input DMAs: SWDGE (GpSimd) with f32->bf16 cast in flight.
        # Issue order = arrival order: batch 0 (coarse pieces) then batch 1
        # (fine pieces, last k chunk split in half). ----
        qt, kt = [], []
        piece_plans = [
            # (q pieces, k pieces, k half-split?)
            ([(0, 4)], [(0, 3), (3, 4)], False),
            ([(0, 2), (2, 4)], [(0, 2), (2, 3)], True),
        ]
        for b in range(BPC):
            t0 = b * TPB
            qp, kp, khalf = piece_plans[b]
            sdt = BF16 if BF16_STREAM else F32
            qt_b = main.tile([P, TPB, HD], sdt, tag=f"qt{b}")
            kt_b = main.tile([P, TPB, HD], sdt, tag=f"kt{b}")
            deng = nc.gpsimd if BF16_STREAM else nc.sync
            for lo, hi in qp:
                deng.dma_start(
                    out=qt_b[:, lo:hi, :],
                    in_=q3[t0 + lo : t0 + hi].rearrange("t p m -> p t m"),
                )
            for lo, hi in kp:
                deng.dma_start(
                    out=kt_b[:, lo:hi, :],
                    in_=k3[t0 + lo : t0 + hi].rearrange("t p m -> p t m"),
                )
            if khalf:
                for c0, c1 in [(0, HD // 2), (HD // 2, HD)]:
                    deng.dma_start(
                        out=kt_b[:, 3:4, c0:c1],
                        in_=k3[t0 + 3 : t0 + 4, :, c0:c1].rearrange(
                            "t p m -> p t m"
                        ),
                    )
            qt.append(qt_b)
            kt.append(kt_b)

        sq = [small.tile([P, TPB * H], F32, name=f"sq{b}", tag=f"sq{b}") for b in range(BPC)]
        sk = [small.tile([P, TPB * H], F32, name=f"sk{b}", tag=f"sk{b}") for b in range(BPC)]
        prod = [small.tile([P, TPB * H], F32, name=f"prod{b}", tag=f"prod{b}") for b in range(BPC)]
        corr_c = [small.tile([P, TPB], F32, name=f"corrc{b}", tag=f"corrc{b}") for b in range(BPC)]
        corr_ps = [psum.tile([1, L], F32, name=f"corrps{b}", tag=f"corrps{b}") for b in range(BPC)]
        maxv = [small.tile([1, 8], F32, name=f"maxv{b}", tag=f"maxv{b}") for b in range(BPC)]
        tu = [small.tile([32, 32], U32, name=f"tu{b}", tag=f"tu{b}") for b in range(BPC)]
        te = [small.tile([32, 32], F32, name=f"te{b}", tag=f"te{b}") for b in range(BPC)]
        teb = [small.tile([KTOP, 1], BF16 if BF16_GATHER else F32, name=f"teb{b}", tag=f"teb{b}") for b in range(BPC)]
        gath = [small.tile([KTOP, HD], BF16 if BF16_GATHER else F32, name=f"g{b}", tag=f"g{b}") for b in range(BPC)]
        s_t = [small.tile([1, 1], F32, name=f"s{b}", tag=f"s{b}") for b in range(BPC)]
        rs = [small.tile([1, 1], F32, name=f"rs{b}", tag=f"rs{b}") for b in range(BPC)]
        mm_ps = [psum.tile([1, HD], F32, name=f"mmps{b}", tag=f"mmps{b}") for b in range(BPC)]
        outt = [small.tile([1, HD], F32, name=f"outt{b}", tag=f"outt{b}") for b in range(BPC)]

        def rowsum(b, s_, t_, lo, hi):
            nc.vector.reduce_sum(
                out=s_[b][:, lo * H : hi * H],
                in_=t_[b][:, lo:hi, :].rearrange("p t (h d) -> p (t h) d", d=D),
                axis=mybir.AxisListType.X,
            )

        def rowsum_half(b, half):
            c0 = half * (HD // 2)
            nc.vector.reduce_sum(
                out=sk[b][:, 3 * H + 4 * half : 3 * H + 4 * (half + 1)],
                in_=kt[b][:, 3:4, c0 : c0 + HD // 2].rearrange(
                    "p t (h d) -> p (t h) d", d=D
                ),
                axis=mybir.AxisListType.X,
            )

        def corr_chunk(b, c):
            # fused: prod = sq*sk, corr_c[:, c] = sum(prod) in one DVE op,
            # then PE transposes the column into the [1, 512] PSUM corr row.
            if USE_TTR:
                nc.vector.tensor_tensor_reduce(
                    out=prod[b][:, c * H : (c + 1) * H],
                    in0=sq[b][:, c * H : (c + 1) * H],
                    in1=sk[b][:, c * H : (c + 1) * H],
                    scale=1.0,
                    scalar=0.0,
                    op0=mybir.AluOpType.mult,
                    op1=mybir.AluOpType.add,
                    accum_out=corr_c[b][:, c : c + 1],
                )
            else:
                nc.vector.tensor_mul(
                    prod[b][:, c * H : (c + 1) * H],
                    sq[b][:, c * H : (c + 1) * H],
                    sk[b][:, c * H : (c + 1) * H],
                )
                nc.vector.reduce_sum(
                    out=corr_c[b][:, c : c + 1],
                    in_=prod[b][:, c * H : (c + 1) * H],
                    axis=mybir.AxisListType.X,
                )
            nc.tensor.transpose(
                out=corr_ps[b][0:1, c * P : (c + 1) * P],
                in_=corr_c[b][:, c : c + 1],
                identity=ident[:],
            )

        def tail_head(b):
            nc.vector.max(out=maxv[b][:], in_=corr_ps[b][:])
            nc.vector.max_index(
                out=stage_u[b][0:1, 0:8],
                in_max=maxv[b][:],
                in_values=corr_ps[b][:],
            )
            if not ELEM_OFF and b:
                nc.vector.tensor_scalar_add(
                    stage_u[b][0:1, 0:8], stage_u[b][0:1, 0:8], b * L
                )
            nc.vector.transpose(out=tu[b][:], in_=stage_u[b][:])

        def tail_gather(b):
            nc.gpsimd.indirect_dma_start(
                out=gath[b][:],
                out_offset=None,
                in_=v,
                in_offset=bass.IndirectOffsetOnAxis(
                    ap=tu[b][0:KTOP, 0:1], axis=0
                ),
                element_offset=(b * L * HD) if ELEM_OFF else 0,
            )

        def tail_aux(b):
            # unnormalized softmax weights: e = exp(corr*SCALE); sum for free
            nc.scalar.activation(
                out=stage_e[b][0:1, 0:KTOP],
                in_=maxv[b][0:1, 0:KTOP],
                func=mybir.ActivationFunctionType.Exp,
                scale=SCALE,
                accum_out=s_t[b][:],
            )
            nc.vector.reciprocal(out=rs[b][:], in_=s_t[b][:])
            nc.vector.transpose(out=te[b][:], in_=stage_e[b][:])
            nc.vector.tensor_copy(teb[b][:], te[b][0:KTOP, 0:1])

        def tail_mm(b):
            nc.tensor.matmul(
                out=mm_ps[b][:],
                lhsT=teb[b][:],
                rhs=gath[b][:],
                start=True,
                stop=True,
            )
            nc.scalar.activation(
                out=outt[b][:],
                in_=mm_ps[b][:],
                func=mybir.ActivationFunctionType.Copy,
                scale=rs[b][0:1, 0:1],
            )
            nc.sync.dma_start(out=out[b : b + 1, :], in_=outt[b][:])

        # ---- hand-interleaved emission (per-engine program order) ----
        rowsum(0, sq, qt, 0, 4)          # DVE: batch-0 q (one piece)
        rowsum(0, sk, kt, 0, 3)          # DVE: batch-0 k piece a
        rowsum(0, sk, kt, 3, 4)          # DVE: batch-0 k piece b
        for c in range(TPB):
            corr_chunk(0, c)             # DVE ttr + PE transpose
        tail_head(0)                     # DVE max/find/transpose
        tail_gather(0)                   # GpSimd (queued after input issues)
        rowsum(1, sq, qt, 0, 2)          # DVE: batch-1 q pieces
        rowsum(1, sq, qt, 2, 4)
        tail_aux(0)                      # ACT exp + DVE recip/te/cast
        rowsum(1, sk, kt, 0, 2)          # DVE: batch-1 k as it lands
        rowsum(1, sk, kt, 2, 3)
        corr_chunk(1, 0)
        corr_chunk(1, 1)
        rowsum_half(1, 0)
        rowsum_half(1, 1)
        corr_chunk(1, 2)
        corr_chunk(1, 3)
        tail_head(1)
        tail_gather(1)
        tail_aux(1)
        tail_mm(0)                       # PE MMs last so they never block T1
        tail_mm(1)


def _build_bass():
    import concourse.bacc as bacc

    nc = bacc.Bacc(trn_type="TRN2", target_bir_lowering=False, debug=False)
    q = nc.dram_tensor("q", [ROWS, HD], mybir.dt.float32, kind="ExternalInput").ap()
    k = nc.dram_tensor("k", [ROWS, HD], mybir.dt.float32, kind="ExternalInput").ap()
    v = nc.dram_tensor("v", [ROWS, HD], mybir.dt.float32, kind="ExternalInput").ap()
    out = nc.dram_tensor(
        "out", [BPC, HD], mybir.dt.float32, kind="ExternalOutput"
    ).ap()
    with tile.TileContext(nc) as tc:
        _emit(tc, q, k, v, out)
    nc.compile()
    return nc


def _get_nc():
    if "nc" not in _CACHE:
        _CACHE["nc"] = _build_bass()
    return _CACHE["nc"]


def run_sharded(queries, keys, values, trace=False, **kw):
    """Shard over 8 cores, run, gather. Returns (out [16,8,64], BassKernelResults)."""
    nc = _get_nc()
    q = np.ascontiguousarray(np.asarray(queries, dtype=np.float32))
    k = np.ascontiguousarray(np.asarray(keys, dtype=np.float32))
    v = np.ascontiguousarray(np.asarray(values, dtype=np.float32))
    in_maps = []
    for c in range(NCORES):
        sl = slice(c * BPC, (c + 1) * BPC)
        in_maps.append(
            {
                "q": q[sl].reshape(ROWS, HD),
                "k": k[sl].reshape(ROWS, HD),
                "v": v[sl].reshape(ROWS, HD),
            }
        )
    res = run_bass_kernel_spmd(nc, in_maps, list(range(NCORES)), trace=trace, **kw)
    out = np.empty((B, H, D), dtype=np.float32)
    for c in range(NCORES):
        out[c * BPC : (c + 1) * BPC] = res.results[c]["out"].reshape(BPC, H, D)
    return out, res


def kernel(queries, keys, values, B=None, **_ignored):
    out, _ = run_sharded(queries, keys, values, trace=False)
    return out


# revision 16
# speedup vs baseline: 1.2673x; 1.1592x over previous
"""AutoCorrelation kernel for Trainium2 (Bass/Tile), 8-core data parallel.

Math: the reference computes rfft over the zero-padded head dim (D=64 -> L=512),
multiplies conj(Q)*K, irffts, then MEANS over heads AND the whole lag axis.
Summing a circular correlation over all lags factorizes:
    sum_t corr[t] = (sum_d q[d]) * (sum_d k[d])
so  x_corr_mean[b,l] = 1/(H*L) * sum_h (sum_d q[b,l,h,:]) * (sum_d k[b,l,h,:]).
Then top-6 over l per batch, softmax, weighted sum of values rows -> [B,H,D].

Sharding: batch 16 -> 2 per core across 8 cores, no cross-core communication.

Schedule (one core): inputs stream via SWDGE DMAs that cast f32->bf16 in
flight (same HBM bytes, half the DVE reduce work at 2x 16-bit throughput;
verified: bf16 corr keeps the same top-6 sets with 9e-4 margins).  Batch 0
streams first and its whole pipeline hides under batch 1's streaming; batch
1's k arrives in single chunks with the last chunk split in half so the
final arrival-to-corr step is short.  DVE emission order is hand-interleaved
so batch-0's tail ops sit in the DMA-arrival gaps.  corr chunks are fused
multiply+reduce (tensor_tensor_reduce) then PE [128,1]->[1,128] column
transposes straight into a [1,512] PSUM row read by MAX8/FIND_INDEX8 (no
SBUF rake).  FIND writes indices directly into the uint32 32x32 staging
tile; one stream transpose puts the 6 gather row-ids on partitions; batch
1's +512 row offset rides the indirect DMA's element_offset.  The gather
casts f32->bf16 in flight and the exp-weight column is cast to bf16, so the
weighted sum is one single-pass bf16 matmul with UNNORMALIZED weights (ACT
exp emits its own sum via accum_out; 1/sum is folded into the PSUM->SBUF
output copy's scale).  A warm exp at entry (written to a live-but-unused
staging row so DCE keeps it) forces the ACT exp-table load before HBM
saturates.
"""

import numpy as np

import concourse.bass as bass
import concourse.mybir as mybir
import concourse.tile as tile
from concourse.masks import make_identity
from concourse.bass_utils import run_bass_kernel_spmd

B, L, H, D = 16, 512, 8, 64
HD = H * D                  # 512
NCORES = 8
BPC = B // NCORES           # 2 batches per core
ROWS = BPC * L              # 1024 rows of [HD] per core
P = 128
TPB = L // P                # 4 chunks per batch
KTOP = 6                    # k = int(log(512)) = 6
SCALE = 1.0 / (H * L)

_CACHE = {}

F32 = mybir.dt.float32
U32 = mybir.dt.uint32
BF16 = mybir.dt.bfloat16

import os
BF16_STREAM = os.environ.get("KBF16_STREAM", "0") == "1"   # SWDGE cast input stream
BF16_GATHER = os.environ.get("KBF16_GATHER", "1") == "1"   # bf16 gather + MM
ELEM_OFF = os.environ.get("KELEM_OFF", "1") == "1"         # element_offset on gather1
USE_TTR = os.environ.get("KTTR", "0") == "1"  # WEDGES THE DEVICE - keep off
ACT_OFF = os.environ.get("KACT", "1") == "1"   # offload q head-groups 0-3 to ACT


def _emit(tc, q, k, v, out):
    # out: single [BPC, HD] DRAM AP.
    nc = tc.nc
    from contextlib import ExitStack

    with ExitStack() as ctx:
        main = ctx.enter_context(tc.tile_pool(name="main", bufs=1))
        small = ctx.enter_context(tc.tile_pool(name="small", bufs=1))
        psum = ctx.enter_context(tc.tile_pool(name="psum", bufs=1, space="PSUM"))

        # Staging tiles for the 32x32 transposes, zeroed early on DVE (idle
        # until the first reduce) so transposes never read uninitialized SBUF.
        stage_u, stage_e = [], []
        for b in range(BPC):
            su = small.tile([32, 32], U32, name=f"su{b}", tag=f"su{b}")
            nc.gpsimd.memset(su[:], 0)
            se = small.tile([32, 32], F32, name=f"se{b}", tag=f"se{b}")
            nc.gpsimd.memset(se[:], 0.0)
            stage_u.append(su)
            stage_e.append(se)

        # Warm exp at entry: forces the ACT exp-table load before the input
        # stream saturates HBM.  Output goes to stage_e[0] row 0, cols 24:32
        # (live tile; lands in unused transpose rows, so DCE keeps it).
        nc.scalar.activation(
            out=stage_e[0][0:1, 24:32],
            in_=stage_e[0][0:1, 24:32],
            func=mybir.ActivationFunctionType.Exp,
            scale=0.0,
        )

        ident = small.tile([P, P], F32)
        make_identity(nc, ident[:])

        q3 = q.rearrange("(t p) m -> t p m", p=P)
        k3 = k.rearrange("(t p) m -> t p m", p=P)

        # ---- input DMAs: SWDGE (GpSimd) with f32->bf16 cast in flight.
        # Issue order = arrival order: batch 0 (coarse pieces) then batch 1
        # (fine pieces, last k chunk split in half). ----
        qt, kt = [], []
        piece_plans = [
            # (q pieces, k pieces, k half-split?)
            ([(0, 2), (2, 4)], [(0, 2), (2, 4)], False),
            ([(0, 2), (2, 4)], [(0, 2), (2, 3)], True),
        ]
        for b in range(BPC):
            t0 = b * TPB
            qp, kp, khalf = piece_plans[b]
            sdt = BF16 if BF16_STREAM else F32
            qt_b = main.tile([P, TPB, HD], sdt, tag=f"qt{b}")
            kt_b = main.tile([P, TPB, HD], sdt, tag=f"kt{b}")
            deng = nc.gpsimd if BF16_STREAM else nc.sync
            for lo, hi in qp:
                deng.dma_start(
                    out=qt_b[:, lo:hi, :],
                    in_=q3[t0 + lo : t0 + hi].rearrange("t p m -> p t m"),
                )
            for lo, hi in kp:
                deng.dma_start(
                    out=kt_b[:, lo:hi, :],
                    in_=k3[t0 + lo : t0 + hi].rearrange("t p m -> p t m"),
                )
            if khalf:
                for c0, c1 in [(0, HD // 2), (HD // 2, HD)]:
                    deng.dma_start(
                        out=kt_b[:, 3:4, c0:c1],
                        in_=k3[t0 + 3 : t0 + 4, :, c0:c1].rearrange(
                            "t p m -> p t m"
                        ),
                    )
            qt.append(qt_b)
            kt.append(kt_b)

        sq = [small.tile([P, TPB * H], F32, name=f"sq{b}", tag=f"sq{b}") for b in range(BPC)]
        sk = [small.tile([P, TPB * H], F32, name=f"sk{b}", tag=f"sk{b}") for b in range(BPC)]
        prod = [small.tile([P, TPB * H], F32, name=f"prod{b}", tag=f"prod{b}") for b in range(BPC)]
        corr_c = [small.tile([P, TPB], F32, name=f"corrc{b}", tag=f"corrc{b}") for b in range(BPC)]
        corr_ps = [psum.tile([1, L], F32, name=f"corrps{b}", tag=f"corrps{b}") for b in range(BPC)]
        maxv = [small.tile([1, 8], F32, name=f"maxv{b}", tag=f"maxv{b}") for b in range(BPC)]
        tu = [small.tile([32, 32], U32, name=f"tu{b}", tag=f"tu{b}") for b in range(BPC)]
        te = [small.tile([32, 32], F32, name=f"te{b}", tag=f"te{b}") for b in range(BPC)]
        teb = [small.tile([KTOP, 1], BF16 if BF16_GATHER else F32, name=f"teb{b}", tag=f"teb{b}") for b in range(BPC)]
        gath = [small.tile([KTOP, HD], BF16 if BF16_GATHER else F32, name=f"g{b}", tag=f"g{b}") for b in range(BPC)]
        s_t = [small.tile([1, 1], F32, name=f"s{b}", tag=f"s{b}") for b in range(BPC)]
        rs = [small.tile([1, 1], F32, name=f"rs{b}", tag=f"rs{b}") for b in range(BPC)]
        mm_ps = [psum.tile([1, HD], F32, name=f"mmps{b}", tag=f"mmps{b}") for b in range(BPC)]
        outt = [small.tile([1, HD], F32, name=f"outt{b}", tag=f"outt{b}") for b in range(BPC)]

        def rowsum(b, s_, t_, lo, hi):
            nc.vector.reduce_sum(
                out=s_[b][:, lo * H : hi * H],
                in_=t_[b][:, lo:hi, :].rearrange("p t (h d) -> p (t h) d", d=D),
                axis=mybir.AxisListType.X,
            )

        def rowsum_half(b, half):
            c0 = half * (HD // 2)
            nc.vector.reduce_sum(
                out=sk[b][:, 3 * H + 4 * half : 3 * H + 4 * (half + 1)],
                in_=kt[b][:, 3:4, c0 : c0 + HD // 2].rearrange(
                    "p t (h d) -> p (t h) d", d=D
                ),
                axis=mybir.AxisListType.X,
            )

        # ACT-offloaded q row-sums: heads 0-3 of each chunk go to the Scalar
        # engine as Copy-activations whose accum_out is the per-partition sum;
        # DVE covers heads 4-7 with a strided reduce.  Relieves the saturated
        # DVE at the cost of idle ACT cycles.
        trash = small.tile([P, D], F32)

        def rowsum_split(b, s_, t_, lo, hi):
            if not ACT_OFF:
                rowsum(b, s_, t_, lo, hi)
                return
            for t in range(lo, hi):
                for h in range(H // 2):
                    nc.scalar.activation(
                        out=trash[:],
                        in_=t_[b][:, t, h * D : (h + 1) * D],
                        func=mybir.ActivationFunctionType.Copy,
                        accum_out=s_[b][:, t * H + h : t * H + h + 1],
                    )
            nc.vector.reduce_sum(
                out=s_[b][:].rearrange("p (t h) -> p t h", h=H)[
                    :, lo:hi, H // 2 :
                ],
                in_=t_[b][:, lo:hi, HD // 2 :].rearrange(
                    "p t (h d) -> p t h d", d=D
                ),
                axis=mybir.AxisListType.X,
            )

        def prod_corr(b):
            # whole-batch prod + grouped corr reduce: 2 DVE ops, then 4 PE
            # column transposes into the [1, 512] PSUM corr row.
            nc.vector.tensor_mul(prod[b][:], sq[b][:], sk[b][:])
            nc.vector.reduce_sum(
                out=corr_c[b][:],
                in_=prod[b][:].rearrange("p (t h) -> p t h", h=H),
                axis=mybir.AxisListType.X,
            )
            for c in range(TPB):
                nc.tensor.transpose(
                    out=corr_ps[b][0:1, c * P : (c + 1) * P],
                    in_=corr_c[b][:, c : c + 1],
                    identity=ident[:],
                )

        def tail_head(b):
            nc.vector.max(out=maxv[b][:], in_=corr_ps[b][:])
            nc.vector.max_index(
                out=stage_u[b][0:1, 0:8],
                in_max=maxv[b][:],
                in_values=corr_ps[b][:],
            )
            if not ELEM_OFF and b:
                nc.vector.tensor_scalar_add(
                    stage_u[b][0:1, 0:8], stage_u[b][0:1, 0:8], b * L
                )
            nc.vector.transpose(out=tu[b][:], in_=stage_u[b][:])

        def tail_gather(b):
            nc.gpsimd.indirect_dma_start(
                out=gath[b][:],
                out_offset=None,
                in_=v,
                in_offset=bass.IndirectOffsetOnAxis(
                    ap=tu[b][0:KTOP, 0:1], axis=0
                ),
                element_offset=(b * L * HD) if ELEM_OFF else 0,
            )

        def tail_aux(b):
            # unnormalized softmax weights: e = exp(corr*SCALE); sum for free
            nc.scalar.activation(
                out=stage_e[b][0:1, 0:KTOP],
                in_=maxv[b][0:1, 0:KTOP],
                func=mybir.ActivationFunctionType.Exp,
                scale=SCALE,
                accum_out=s_t[b][:],
            )
            nc.vector.reciprocal(out=rs[b][:], in_=s_t[b][:])
            nc.vector.transpose(out=te[b][:], in_=stage_e[b][:])
            nc.vector.tensor_copy(teb[b][:], te[b][0:KTOP, 0:1])

        def tail_mm(b):
            nc.tensor.matmul(
                out=mm_ps[b][:],
                lhsT=teb[b][:],
                rhs=gath[b][:],
                start=True,
                stop=True,
            )
            nc.scalar.activation(
                out=outt[b][:],
                in_=mm_ps[b][:],
                func=mybir.ActivationFunctionType.Copy,
                scale=rs[b][0:1, 0:1],
            )
            nc.sync.dma_start(out=out[b : b + 1, :], in_=outt[b][:])

        # ---- hand-interleaved emission ----
        rowsum_split(0, sq, qt, 0, 2)    # q0a: ACT heads 0-3, DVE heads 4-7
        rowsum_split(0, sq, qt, 2, 4)    # q0b
        rowsum(0, sk, kt, 0, 2)          # DVE: k0 full
        rowsum(0, sk, kt, 2, 4)
        prod_corr(0)                     # 2 DVE ops + 4 PE transposes
        tail_head(0)                     # DVE max/find/transpose
        tail_gather(0)                   # GpSimd
        rowsum_split(1, sq, qt, 0, 2)    # q1a: ACT/DVE split
        rowsum_split(1, sq, qt, 2, 4)    # q1b
        rowsum(1, sk, kt, 0, 2)          # DVE: k1 as it lands
        rowsum(1, sk, kt, 2, 3)
        rowsum_half(1, 0)
        rowsum_half(1, 1)
        prod_corr(1)
        tail_head(1)
        tail_gather(1)
        tail_aux(0)                      # aux late: keeps DVE clear for k1
        tail_aux(1)
        tail_mm(0)                       # PE MMs last so they never block T1
        tail_mm(1)


def _build_bass():
    import concourse.bacc as bacc

    nc = bacc.Bacc(trn_type="TRN2", target_bir_lowering=False, debug=False)
    q = nc.dram_tensor("q", [ROWS, HD], mybir.dt.float32, kind="ExternalInput").ap()
    k = nc.dram_tensor("k", [ROWS, HD], mybir.dt.float32, kind="ExternalInput").ap()
    v = nc.dram_tensor("v", [ROWS, HD], mybir.dt.float32, kind="ExternalInput").ap()
    out = nc.dram_tensor(
        "out", [BPC, HD], mybir.dt.float32, kind="ExternalOutput"
    ).ap()
    with tile.TileContext(nc) as tc:
        _emit(tc, q, k, v, out)
    nc.compile()
    return nc


def _get_nc():
    if "nc" not in _CACHE:
        _CACHE["nc"] = _build_bass()
    return _CACHE["nc"]


def run_sharded(queries, keys, values, trace=False, **kw):
    """Shard over 8 cores, run, gather. Returns (out [16,8,64], BassKernelResults)."""
    nc = _get_nc()
    q = np.ascontiguousarray(np.asarray(queries, dtype=np.float32))
    k = np.ascontiguousarray(np.asarray(keys, dtype=np.float32))
    v = np.ascontiguousarray(np.asarray(values, dtype=np.float32))
    in_maps = []
    for c in range(NCORES):
        sl = slice(c * BPC, (c + 1) * BPC)
        in_maps.append(
            {
                "q": q[sl].reshape(ROWS, HD),
                "k": k[sl].reshape(ROWS, HD),
                "v": v[sl].reshape(ROWS, HD),
            }
        )
    res = run_bass_kernel_spmd(nc, in_maps, list(range(NCORES)), trace=trace, **kw)
    out = np.empty((B, H, D), dtype=np.float32)
    for c in range(NCORES):
        out[c * BPC : (c + 1) * BPC] = res.results[c]["out"].reshape(BPC, H, D)
    return out, res


def kernel(queries, keys, values, B=None, **_ignored):
    out, _ = run_sharded(queries, keys, values, trace=False)
    return out
